# revision 5
# baseline (speedup 1.0000x reference)
"""Trainium2 Bass kernel for nn_MetaPathGNN (3-layer metapath GNN, N=100000,
E=1000000, H=64, metapath (0,1,2)).

Math (per layer l, rel = l; note x_in == h in every layer of the reference):
    agg[s] = sum_{e: type(e)==l, src(e)==s} h[dst(e)]
    h      = relu(agg @ wl.T + h @ (w0+w1).T + (bl+b0+b1))

Distribution: shard by src-node range (12500 nodes/core, 8 cores). Each core
owns all edges whose src falls in its range, so its agg slice is complete
locally -- no reduction needed. The full node table h is re-replicated between
layers with an AllGather (the only collective).

Per-core pipeline per layer:
  - dma_gather pulls h[dst] rows (f32) from the DRAM node table in tile order
    (128 edges/tile). Tables are addressed in 4 chunks of 25000 rows because
    gather indices are int16.
  - A selection matrix S[e, n] = (src_local[e] == n) is built per 128-node
    block with one iota-vs-index is_equal, and the segment sum is computed as
    PSUM-accumulating matmuls: agg_T[h, n] += msg_e[:,h].T @ S.
  - The two linear terms + bias are more PSUM matmuls, in both feature-major
    (next layer's h-term operand) and node-major (gather table / output)
    orientations, followed by a fused ReLU(+bias) activation.
"""
import os
import sys
import time

import numpy as np

for _p in ("/opt/trn_rl_repo", os.path.expanduser("~/.axon_site/_ro/trn_rl_repo")):
    if os.path.isdir(_p) and _p not in sys.path:
        sys.path.insert(0, _p)

import concourse.bass as bass
import concourse.tile as tile
from concourse import bacc, mybir
from concourse.bass_utils import run_bass_kernel_spmd

F32 = mybir.dt.float32
I16 = mybir.dt.int16
I32 = mybir.dt.int32

N = 100000
E = 1000000
H = 64
NC = 8
NPC = N // NC          # 12500 nodes per core
B = 128                # nodes per block
NBLK = -(-NPC // B)    # 98 (last block 84 nodes)
CH = 25000             # gather-table chunk rows (int16-addressable)
NK = N // CH           # 4 chunks
WB = 16                # blocks per wave
H_IN = [2 * H, H, H]   # per-layer input feature dim


# --------------------------------------------------------------------------
# host-side preprocessing
# --------------------------------------------------------------------------

def preprocess_edges(edge_index, edge_type):
    """Returns (meta, per_core) where meta is the SPMD-uniform tile layout and
    per_core[c] = {"idx{l}": int16 [128, P_l/16], "ls{l}": f32 [128, T_l]}."""
    src = np.asarray(edge_index[0]).astype(np.int64)
    dst = np.asarray(edge_index[1]).astype(np.int64)
    et = np.asarray(edge_type).astype(np.int64)

    meta = {"layers": []}
    per_core = [dict() for _ in range(NC)]

    for l in range(3):
        sel = et == l
        s, d = src[sel], dst[sel]
        core = s // NPC
        blk = (s % NPC) // B
        chk = d // CH
        cell = (core * NBLK + blk) * NK + chk
        order = np.argsort(cell, kind="stable")
        cell_s = cell[order]
        drel_s = (d - chk * CH)[order].astype(np.int16)
        ls_s = ((s % NPC) % B)[order].astype(np.float32)
        bounds = np.searchsorted(cell_s, np.arange(NC * NBLK * NK + 1))
        counts = np.diff(bounds).reshape(NC, NBLK, NK)
        tiles_cell = -(-counts.max(axis=0) // 128)      # [NBLK, NK]

        # wave / call / block layout (identical across cores)
        waves = []
        g = 0                                            # global gather tile idx
        gtile_of_cell = np.zeros((NBLK, NK), np.int64)
        for w0 in range(0, NBLK, WB):
            nb = min(WB, NBLK - w0)
            wave = {"b0": w0, "nb": nb, "G0": g, "calls": [], "blocks": []}
            for k in range(NK):
                twk = int(tiles_cell[w0:w0 + nb, k].sum())
                if twk:
                    wave["calls"].append((k, g, twk))
                for b in range(w0, w0 + nb):
                    gtile_of_cell[b, k] = g + int(tiles_cell[w0:b, k].sum())
                g += twk
            wave["T"] = g - wave["G0"]
            waves.append(wave)
        T_total = g

        stile_of_cell = np.zeros((NBLK, NK), np.int64)
        scol = np.zeros(NBLK + 1, np.int64)
        for b in range(NBLK):
            scol[b + 1] = scol[b] + int(tiles_cell[b].sum())
            stile_of_cell[b] = scol[b] + np.concatenate(
                ([0], np.cumsum(tiles_cell[b][:-1])))
        assert scol[NBLK] == T_total

        for wave in waves:
            for b in range(wave["b0"], wave["b0"] + wave["nb"]):
                tiles = []
                for k in range(NK):
                    for t in range(int(tiles_cell[b, k])):
                        tiles.append(int(gtile_of_cell[b, k]) + t - wave["G0"])
                bw = min(B, NPC - b * B)
                wave["blocks"].append(
                    {"b": b, "bw": bw, "scol": int(scol[b]), "tiles": tiles})

        meta["layers"].append({"T": T_total, "waves": waves})

        # per-core padded index / local-src arrays
        cellbase_g = (gtile_of_cell.reshape(-1) * 128)[None, :] \
            + np.zeros((NC, 1), np.int64)                # same layout all cores
        cellbase_s = (stile_of_cell.reshape(-1) * 128)[None, :] \
            + np.zeros((NC, 1), np.int64)
        rank = np.arange(cell_s.shape[0]) - bounds[cell_s]
        flatcell = cell_s                                 # (c*NBLK+b)*NK+k
        cbk = flatcell % (NBLK * NK)
        gdst = cellbase_g[0][cbk] + rank
        sdst = cellbase_s[0][cbk] + rank
        core_s = flatcell // (NBLK * NK)
        for c in range(NC):
            m = core_s == c
            gv = np.zeros(T_total * 128, np.int16)
            gv[gdst[m]] = drel_s[m]
            lv = np.full(T_total * 128, -1.0, np.float32)
            lv[sdst[m]] = ls_s[m]
            per_core[c][f"idx{l}"] = np.tile(
                gv.reshape(-1, 16).T, (8, 1)).copy()      # [128, T*8]
            per_core[c][f"ls{l}"] = lv.reshape(T_total, 128).T.copy()

    return meta, per_core


def prep_weights(inputs):
    out = {}
    for l in range(3):
        wl = np.asarray(inputs[f"wl{l}"], np.float32)
        wc = np.asarray(inputs[f"w0{l}"], np.float32) \
            + np.asarray(inputs[f"w1{l}"], np.float32)
        bias = np.asarray(inputs[f"wbl{l}"], np.float32) \
            + np.asarray(inputs[f"wb0{l}"], np.float32) \
            + np.asarray(inputs[f"wb1{l}"], np.float32)
        out[f"wlT{l}"] = np.ascontiguousarray(wl.T)       # [H_in, 64]
        out[f"wcT{l}"] = np.ascontiguousarray(wc.T)       # [H_in, 64]
        out[f"bias{l}"] = bias                            # [64]
    return out


# --------------------------------------------------------------------------
# kernel builder
# --------------------------------------------------------------------------

def build_nc(meta):
    nc = bacc.Bacc("TRN2", target_bir_lowering=False, debug=False,
                   num_devices=NC)
    xtab = nc.declare_dram_parameter("xtab", [N, 2 * H], F32, isOutput=False)
    xT = nc.declare_dram_parameter("xT", [2 * H, NPC], F32, isOutput=False)
    prm = {"xtab": xtab, "xT": xT}
    for l in range(3):
        T = meta["layers"][l]["T"]
        prm[f"idx{l}"] = nc.declare_dram_parameter(
            f"idx{l}", [128, T * 8], I16, isOutput=False)
        prm[f"ls{l}"] = nc.declare_dram_parameter(
            f"ls{l}", [128, T], F32, isOutput=False)
        prm[f"wlT{l}"] = nc.declare_dram_parameter(
            f"wlT{l}", [H_IN[l], H], F32, isOutput=False)
        prm[f"wcT{l}"] = nc.declare_dram_parameter(
            f"wcT{l}", [H_IN[l], H], F32, isOutput=False)
        prm[f"bias{l}"] = nc.declare_dram_parameter(
            f"bias{l}", [H], F32, isOutput=False)
    out = nc.declare_dram_parameter("out", [NPC, H], F32, isOutput=True)

    htab = [xtab]
    hslc, hT_d = [None], [xT]
    for l in (1, 2):
        hslc.append(nc.dram_tensor(f"h{l}_slice", [NPC, H], F32))
        htab.append(nc.dram_tensor(f"h{l}_tab", [N, H], F32,
                                   addr_space="Shared"))
        hT_d.append(nc.dram_tensor(f"hT{l}", [H, NPC], F32))

    relu = mybir.ActivationFunctionType.Relu

    with tile.TileContext(nc) as tc:
        with tc.tile_pool(name="const", bufs=1) as cpool, \
             tc.tile_pool(name="idx", bufs=1) as ipool, \
             tc.tile_pool(name="msg", bufs=2) as mpool, \
             tc.tile_pool(name="sgen", bufs=2) as spool, \
             tc.tile_pool(name="hio", bufs=2) as hpool, \
             tc.tile_pool(name="blk", bufs=3) as bpool, \
             tc.tile_pool(name="psA", bufs=2, space="PSUM") as psA, \
             tc.tile_pool(name="psB", bufs=2, space="PSUM") as psB, \
             tc.tile_pool(name="psC", bufs=2, space="PSUM") as psC:

            iota_i = cpool.tile([128, B], I32)
            nc.gpsimd.iota(iota_i[:], pattern=[[1, B]], base=0,
                           channel_multiplier=0)
            iota_f = cpool.tile([128, B], F32)
            nc.vector.tensor_copy(iota_f[:], iota_i[:])
            ones_sb = cpool.tile([1, B], F32)
            nc.vector.memset(ones_sb[:], 1.0)

            wsb = {}
            for l in range(3):
                wsb[f"wlT{l}"] = cpool.tile([H_IN[l], H], F32, tag=f"wlT{l}", name=f"wlT{l}")
                nc.sync.dma_start(wsb[f"wlT{l}"][:], prm[f"wlT{l}"][:])
                wsb[f"wcT{l}"] = cpool.tile([H_IN[l], H], F32, tag=f"wcT{l}", name=f"wcT{l}")
                nc.sync.dma_start(wsb[f"wcT{l}"][:], prm[f"wcT{l}"][:])
                wsb[f"bc{l}"] = cpool.tile([H, 1], F32, tag=f"bc{l}", name=f"bc{l}")
                nc.sync.dma_start(wsb[f"bc{l}"][:],
                                  prm[f"bias{l}"][:].unsqueeze(1))
                wsb[f"br{l}"] = cpool.tile([1, H], F32, tag=f"br{l}", name=f"br{l}")
                nc.sync.dma_start(wsb[f"br{l}"][:],
                                  prm[f"bias{l}"][:].unsqueeze(0))

            for l in range(3):
                hin = H_IN[l]
                L = meta["layers"][l]
                T = L["T"]
                idx_sb = ipool.tile([128, T * 8], I16, tag=f"idx{l}")
                nc.sync.dma_start(idx_sb[:], prm[f"idx{l}"][:])
                ls_sb = ipool.tile([128, T], F32, tag=f"ls{l}")
                nc.sync.dma_start(ls_sb[:], prm[f"ls{l}"][:])
                wlT, wcT = wsb[f"wlT{l}"], wsb[f"wcT{l}"]
                bc, br = wsb[f"bc{l}"], wsb[f"br{l}"]

                for wave in L["waves"]:
                    Tw, G0 = wave["T"], wave["G0"]
                    nb, b0 = wave["nb"], wave["b0"]
                    wnodes = min(nb * B, NPC - b0 * B)
                    msg = mpool.tile([128, Tw * hin], F32, tag="msg")
                    for (k, g0, twk) in wave["calls"]:
                        nc.gpsimd.dma_gather(
                            out_ap=msg[:, (g0 - G0) * hin:(g0 - G0 + twk) * hin]
                                .rearrange("p (t e) -> p t e", e=hin),
                            in_ap=htab[l][k * CH:(k + 1) * CH, :],
                            idxs_ap=idx_sb[:, g0 * 8:(g0 + twk) * 8],
                            num_idxs=twk * 128,
                            num_idxs_reg=twk * 128,
                            elem_size=hin,
                            single_packet=(twk * 128 <= 1024),
                        )
                    hT_in = hpool.tile([hin, nb * B], F32, tag="hT_in")
                    nc.sync.dma_start(hT_in[:, :wnodes],
                                      hT_d[l][:, b0 * B:b0 * B + wnodes])

                    for blkm in wave["blocks"]:
                        b, bw = blkm["b"], blkm["bw"]
                        tb = len(blkm["tiles"])
                        sc = blkm["scol"]
                        hTb = hT_in[:, (b - b0) * B:(b - b0) * B + bw]

                        agg_sb = bpool.tile([hin, B], F32, tag="agg")
                        if tb:
                            S = spool.tile([128, tb * B], F32, tag="S")
                            nc.vector.tensor_tensor(
                                out=S[:].rearrange("p (t b) -> p t b", b=B),
                                in0=ls_sb[:, sc:sc + tb].unsqueeze(2)
                                    .broadcast_to([128, tb, B]),
                                in1=iota_f[:].unsqueeze(1)
                                    .broadcast_to([128, tb, B]),
                                op=mybir.AluOpType.is_equal,
                            )
                            agg_ps = psA.tile([hin, B], F32, space="PSUM",
                                              tag="aggps")
                            for j, slot in enumerate(blkm["tiles"]):
                                nc.tensor.matmul(
                                    out=agg_ps[:],
                                    lhsT=msg[:, slot * hin:(slot + 1) * hin],
                                    rhs=S[:, j * B:(j + 1) * B],
                                    start=(j == 0), stop=(j == tb - 1),
                                )
                            nc.scalar.copy(agg_sb[:], agg_ps[:])
                        else:
                            nc.vector.memset(agg_sb[:], 0.0)

                        # node-major: [bw, H] = agg.T@wlT + hT.T@wcT + 1.T@bias
                        hN_ps = psC.tile([B, H], F32, space="PSUM", tag="hN")
                        nc.tensor.matmul(out=hN_ps[:bw, :],
                                         lhsT=agg_sb[:, :bw],
                                         rhs=wlT[:], start=True, stop=False)
                        nc.tensor.matmul(out=hN_ps[:bw, :], lhsT=hTb,
                                         rhs=wcT[:], start=False, stop=False)
                        nc.tensor.matmul(out=hN_ps[:bw, :],
                                         lhsT=ones_sb[:, :bw],
                                         rhs=br[:], start=False, stop=True)
                        hN_sb = bpool.tile([B, H], F32, tag="hN_sb")
                        nc.scalar.activation(hN_sb[:bw, :], hN_ps[:bw, :],
                                             relu)
                        dst_d = out if l == 2 else hslc[l + 1]
                        nc.sync.dma_start(dst_d[b * B:b * B + bw, :],
                                          hN_sb[:bw, :])

                        if l < 2:
                            hT_ps = psB.tile([H, B], F32, space="PSUM",
                                             tag="hT")
                            nc.tensor.matmul(out=hT_ps[:, :bw],
                                             lhsT=wlT[:],
                                             rhs=agg_sb[:, :bw], start=True,
                                             stop=False)
                            nc.tensor.matmul(out=hT_ps[:, :bw], lhsT=wcT[:],
                                             rhs=hTb, start=False, stop=True)
                            hT_sb = bpool.tile([H, B], F32, tag="hT_sb")
                            nc.scalar.activation(hT_sb[:, :bw],
                                                 hT_ps[:, :bw], relu,
                                                 bias=bc[:])
                            nc.sync.dma_start(
                                hT_d[l + 1][:, b * B:b * B + bw],
                                hT_sb[:, :bw])

                if l < 2:
                    nc.gpsimd.collective_compute(
                        "AllGather",
                        mybir.AluOpType.bypass,
                        ins=[hslc[l + 1][:]],
                        outs=[htab[l + 1][:]],
                        replica_groups=[list(range(NC))],
                    )

    nc.compile()
    return nc


# --------------------------------------------------------------------------
# public entry point
# --------------------------------------------------------------------------

_CACHE = {}


def _prepare(inputs):
    t0 = time.time()
    meta, per_core = preprocess_edges(np.asarray(inputs["edge_index"]),
                                      np.asarray(inputs["edge_type"]))
    w = prep_weights(inputs)
    x = np.ascontiguousarray(np.asarray(inputs["x"], np.float32))
    xTf = np.ascontiguousarray(x.T)
    in_maps = []
    for c in range(NC):
        m = {"xtab": x, "xT": np.ascontiguousarray(
            xTf[:, c * NPC:(c + 1) * NPC])}
        m.update(w)
        m.update(per_core[c])
        in_maps.append(m)
    key = tuple(meta["layers"][l]["T"] for l in range(3))
    if key not in _CACHE:
        _CACHE[key] = build_nc(meta)
    return _CACHE[key], in_maps, time.time() - t0


def run(inputs, **kw):
    nc, in_maps, _ = _prepare(inputs)
    res = run_bass_kernel_spmd(nc, in_maps, list(range(NC)), **kw)
    full = np.concatenate([res.results[c]["out"] for c in range(NC)], axis=0)
    return full, res


def kernel(**inputs) -> np.ndarray:
    full, _ = run(inputs)
    return full


# revision 6
# speedup vs baseline: 1.0458x; 1.0458x over previous
"""Trainium2 Bass kernel for nn_MetaPathGNN (3-layer metapath GNN, N=100000,
E=1000000, H=64, metapath (0,1,2)).

Math (per layer l, rel = l; note x_in == h in every layer of the reference):
    agg[s] = sum_{e: type(e)==l, src(e)==s} h[dst(e)]
    h      = relu(agg @ wl.T + h @ (w0+w1).T + (bl+b0+b1))

Distribution: shard by src-node range (12500 nodes/core, 8 cores). Each core
owns all edges whose src falls in its range, so its agg slice is complete
locally -- no reduction needed. The gather table shipped between layers holds
PRE-MULTIPLIED rows hw = h @ wl.T (so the edge-segment matmul directly yields
agg @ wl.T), replicated with an AllGather per layer.

Per-core pipeline per layer:
  - dma_gather pulls hw[dst] rows (64 f32 = 256 B) from the DRAM table in
    tile order (128 edges/tile), 4 table chunks of 25000 rows (int16 idx).
  - A selection matrix S[e, n] = (src_local[e] == n) is built per 128-node
    block with one fused iota-vs-index is_equal; segment sums are
    PSUM-accumulating matmuls against S.
  - The h-term and bias fold into the same PSUM group; ReLU(+bias) via the
    scalar engine. Outputs are produced feature-major (next layer's h-term)
    and as the next pre-multiplied table slice.
"""
import os
import sys
import time

import numpy as np

for _p in ("/opt/trn_rl_repo", os.path.expanduser("~/.axon_site/_ro/trn_rl_repo")):
    if os.path.isdir(_p) and _p not in sys.path:
        sys.path.insert(0, _p)

import concourse.bass as bass
import concourse.tile as tile
from concourse import bacc, mybir
from concourse.bass_utils import run_bass_kernel_spmd

F32 = mybir.dt.float32
I16 = mybir.dt.int16
I32 = mybir.dt.int32

N = 100000
E = 1000000
H = 64
NC = 8
NPC = N // NC          # 12500 nodes per core
B = 128                # nodes per block
NBLK = -(-NPC // B)    # 98 (last block 84 nodes)
CH = 25000             # gather-table chunk rows (int16-addressable)
NK = N // CH           # 4 chunks
WB = 32                # blocks per wave
H_IN = [2 * H, H, H]   # per-layer h-term feature dim


# --------------------------------------------------------------------------
# host-side preprocessing
# --------------------------------------------------------------------------

def preprocess_edges(edge_index, edge_type):
    """Returns (meta, per_core) where meta is the SPMD-uniform tile layout and
    per_core[c] = {"idx{l}": int16 [128, T_l*8], "ls{l}": f32 [128, T_l]}."""
    src = np.asarray(edge_index[0]).astype(np.int64)
    dst = np.asarray(edge_index[1]).astype(np.int64)
    et = np.asarray(edge_type).astype(np.int64)

    meta = {"layers": []}
    per_core = [dict() for _ in range(NC)]

    for l in range(3):
        sel = et == l
        s, d = src[sel], dst[sel]
        core = s // NPC
        blk = (s % NPC) // B
        chk = d // CH
        cell = (core * NBLK + blk) * NK + chk
        order = np.argsort(cell, kind="stable")
        cell_s = cell[order]
        drel_s = (d - chk * CH)[order].astype(np.int16)
        ls_s = ((s % NPC) % B)[order].astype(np.float32)
        bounds = np.searchsorted(cell_s, np.arange(NC * NBLK * NK + 1))
        counts = np.diff(bounds).reshape(NC, NBLK, NK)
        tiles_cell = -(-counts.max(axis=0) // 128)      # [NBLK, NK]

        # wave / call / block layout (identical across cores)
        waves = []
        g = 0                                            # global gather tile idx
        gtile_of_cell = np.zeros((NBLK, NK), np.int64)
        for w0 in range(0, NBLK, WB):
            nb = min(WB, NBLK - w0)
            wave = {"b0": w0, "nb": nb, "G0": g, "calls": [], "blocks": []}
            for k in range(NK):
                twk = int(tiles_cell[w0:w0 + nb, k].sum())
                if twk:
                    wave["calls"].append((k, g, twk))
                for b in range(w0, w0 + nb):
                    gtile_of_cell[b, k] = g + int(tiles_cell[w0:b, k].sum())
                g += twk
            wave["T"] = g - wave["G0"]
            waves.append(wave)
        T_total = g

        stile_of_cell = np.zeros((NBLK, NK), np.int64)
        scol = np.zeros(NBLK + 1, np.int64)
        for b in range(NBLK):
            scol[b + 1] = scol[b] + int(tiles_cell[b].sum())
            stile_of_cell[b] = scol[b] + np.concatenate(
                ([0], np.cumsum(tiles_cell[b][:-1])))
        assert scol[NBLK] == T_total

        for wave in waves:
            for b in range(wave["b0"], wave["b0"] + wave["nb"]):
                tiles = []
                for k in range(NK):
                    for t in range(int(tiles_cell[b, k])):
                        tiles.append(int(gtile_of_cell[b, k]) + t - wave["G0"])
                bw = min(B, NPC - b * B)
                wave["blocks"].append(
                    {"b": b, "bw": bw, "scol": int(scol[b]), "tiles": tiles})

        meta["layers"].append({"T": T_total, "waves": waves})

        # per-core padded index / local-src arrays
        rank = np.arange(cell_s.shape[0]) - bounds[cell_s]
        cbk = cell_s % (NBLK * NK)
        gdst = (gtile_of_cell.reshape(-1) * 128)[cbk] + rank
        sdst = (stile_of_cell.reshape(-1) * 128)[cbk] + rank
        core_s = cell_s // (NBLK * NK)
        for c in range(NC):
            m = core_s == c
            gv = np.zeros(T_total * 128, np.int16)
            gv[gdst[m]] = drel_s[m]
            lv = np.full(T_total * 128, -1.0, np.float32)
            lv[sdst[m]] = ls_s[m]
            per_core[c][f"idx{l}"] = np.tile(
                gv.reshape(-1, 16).T, (8, 1)).copy()      # [128, T*8]
            per_core[c][f"ls{l}"] = lv.reshape(T_total, 128).T.copy()

    return meta, per_core


def prep_weights(inputs):
    out = {}
    for l in range(3):
        wl = np.asarray(inputs[f"wl{l}"], np.float32)
        wc = np.asarray(inputs[f"w0{l}"], np.float32) \
            + np.asarray(inputs[f"w1{l}"], np.float32)
        bias = np.asarray(inputs[f"wbl{l}"], np.float32) \
            + np.asarray(inputs[f"wb0{l}"], np.float32) \
            + np.asarray(inputs[f"wb1{l}"], np.float32)
        out[f"wlT{l}"] = np.ascontiguousarray(wl.T)       # [H_in, 64]
        out[f"wcT{l}"] = np.ascontiguousarray(wc.T)       # [H_in, 64]
        out[f"bias{l}"] = bias                            # [64]
    return out


# --------------------------------------------------------------------------
# kernel builder
# --------------------------------------------------------------------------

def build_nc(meta):
    nc = bacc.Bacc("TRN2", target_bir_lowering=False, debug=False,
                   num_devices=NC)
    xT = nc.declare_dram_parameter("xT", [2 * H, NPC], F32, isOutput=False)
    prm = {"xT": xT}
    for l in range(3):
        T = meta["layers"][l]["T"]
        prm[f"idx{l}"] = nc.declare_dram_parameter(
            f"idx{l}", [128, T * 8], I16, isOutput=False)
        prm[f"ls{l}"] = nc.declare_dram_parameter(
            f"ls{l}", [128, T], F32, isOutput=False)
        prm[f"wlT{l}"] = nc.declare_dram_parameter(
            f"wlT{l}", [H_IN[l], H], F32, isOutput=False)
        prm[f"wcT{l}"] = nc.declare_dram_parameter(
            f"wcT{l}", [H_IN[l], H], F32, isOutput=False)
        prm[f"bias{l}"] = nc.declare_dram_parameter(
            f"bias{l}", [H], F32, isOutput=False)
    out = nc.declare_dram_parameter("out", [NPC, H], F32, isOutput=True)

    # internal DRAM: per-layer gather tables (hw = h @ wl.T) + h-term sources
    slc, tab, hT_d = [], [], [xT]
    for l in range(3):
        slc.append(nc.dram_tensor(f"hw{l}_slice", [NPC, H], F32))
        tab.append(nc.dram_tensor(f"hw{l}_tab", [N, H], F32,
                                  addr_space="Shared"))
        if l < 2:
            hT_d.append(nc.dram_tensor(f"hT{l + 1}", [H, NPC], F32))

    relu = mybir.ActivationFunctionType.Relu
    rg = [list(range(NC))]
    wave_sched = [(min(WB, NBLK - w0), w0) for w0 in range(0, NBLK, WB)]

    with tile.TileContext(nc) as tc:
        with tc.tile_pool(name="const", bufs=1) as cpool, \
             tc.tile_pool(name="idx", bufs=1) as ipool, \
             tc.tile_pool(name="msg", bufs=2) as mpool, \
             tc.tile_pool(name="sgen", bufs=2) as spool, \
             tc.tile_pool(name="hio", bufs=2) as hpool, \
             tc.tile_pool(name="blk", bufs=3) as bpool, \
             tc.tile_pool(name="psB", bufs=2, space="PSUM") as psB, \
             tc.tile_pool(name="psC", bufs=2, space="PSUM") as psC, \
             tc.tile_pool(name="psD", bufs=2, space="PSUM") as psD:

            iota_i = cpool.tile([128, B], I32)
            nc.gpsimd.iota(iota_i[:], pattern=[[1, B]], base=0,
                           channel_multiplier=0)
            iota_f = cpool.tile([128, B], F32)
            nc.vector.tensor_copy(iota_f[:], iota_i[:])
            ones_sb = cpool.tile([1, B], F32)
            nc.vector.memset(ones_sb[:], 1.0)

            wsb = {}
            for l in range(3):
                wsb[f"wlT{l}"] = cpool.tile([H_IN[l], H], F32,
                                            tag=f"wlT{l}", name=f"wlT{l}")
                nc.sync.dma_start(wsb[f"wlT{l}"][:], prm[f"wlT{l}"][:])
                wsb[f"wcT{l}"] = cpool.tile([H_IN[l], H], F32,
                                            tag=f"wcT{l}", name=f"wcT{l}")
                nc.sync.dma_start(wsb[f"wcT{l}"][:], prm[f"wcT{l}"][:])
                wsb[f"bc{l}"] = cpool.tile([H, 1], F32, tag=f"bc{l}",
                                           name=f"bc{l}")
                nc.sync.dma_start(wsb[f"bc{l}"][:],
                                  prm[f"bias{l}"][:].unsqueeze(1))
                wsb[f"br{l}"] = cpool.tile([1, H], F32, tag=f"br{l}",
                                           name=f"br{l}")
                nc.sync.dma_start(wsb[f"br{l}"][:],
                                  prm[f"bias{l}"][:].unsqueeze(0))

            # ---- prologue: hw0 = x @ wl0.T (own slice), then AllGather ----
            for nb, b0 in wave_sched:
                wnodes = min(nb * B, NPC - b0 * B)
                xT_in = hpool.tile([2 * H, WB * B], F32, tag="hT_in")
                nc.sync.dma_start(xT_in[:, :wnodes],
                                  xT[:, b0 * B:b0 * B + wnodes])
                for bi in range(nb):
                    b = b0 + bi
                    bw = min(B, NPC - b * B)
                    xw_ps = psD.tile([B, H], F32, space="PSUM", tag="xw")
                    nc.tensor.matmul(out=xw_ps[:bw, :],
                                     lhsT=xT_in[:, bi * B:bi * B + bw],
                                     rhs=wsb["wlT0"][:],
                                     start=True, stop=True)
                    xw_sb = bpool.tile([B, H], F32, tag="hw_sb")
                    nc.scalar.copy(xw_sb[:bw, :], xw_ps[:bw, :])
                    nc.sync.dma_start(slc[0][b * B:b * B + bw, :],
                                      xw_sb[:bw, :])
            nc.gpsimd.collective_compute(
                "AllGather", mybir.AluOpType.bypass,
                ins=[slc[0][:]], outs=[tab[0][:]], replica_groups=rg)

            # ---- layers ----
            for l in range(3):
                hin = H_IN[l]
                L = meta["layers"][l]
                T = L["T"]
                idx_sb = ipool.tile([128, T * 8], I16, tag=f"idx{l}",
                                    name=f"idx{l}")
                nc.sync.dma_start(idx_sb[:], prm[f"idx{l}"][:])
                ls_sb = ipool.tile([128, T], F32, tag=f"ls{l}",
                                   name=f"ls{l}")
                nc.sync.dma_start(ls_sb[:], prm[f"ls{l}"][:])
                wlT_nxt = wsb[f"wlT{l + 1}"] if l < 2 else None
                wcT = wsb[f"wcT{l}"]
                bc, br = wsb[f"bc{l}"], wsb[f"br{l}"]

                for wave in L["waves"]:
                    Tw, G0 = wave["T"], wave["G0"]
                    nb, b0 = wave["nb"], wave["b0"]
                    wnodes = min(nb * B, NPC - b0 * B)
                    msg = mpool.tile([128, Tw * H], F32, tag="msg")
                    for (k, g0, twk) in wave["calls"]:
                        nc.gpsimd.dma_gather(
                            out_ap=msg[:, (g0 - G0) * H:(g0 - G0 + twk) * H]
                                .rearrange("p (t e) -> p t e", e=H),
                            in_ap=tab[l][k * CH:(k + 1) * CH, :],
                            idxs_ap=idx_sb[:, g0 * 8:(g0 + twk) * 8],
                            num_idxs=twk * 128,
                            num_idxs_reg=twk * 128,
                            elem_size=H,
                            single_packet=(twk * 128 <= 1024),
                        )
                    hT_in = hpool.tile([hin, WB * B], F32, tag="hT_in")
                    nc.sync.dma_start(hT_in[:, :wnodes],
                                      hT_d[l][:, b0 * B:b0 * B + wnodes])

                    for blkm in wave["blocks"]:
                        b, bw = blkm["b"], blkm["bw"]
                        tb = len(blkm["tiles"])
                        sc = blkm["scol"]
                        hTb = hT_in[:, (b - b0) * B:(b - b0) * B + bw]

                        S = None
                        if tb:
                            S = spool.tile([128, tb * B], F32, tag="S")
                            nc.vector.tensor_tensor(
                                out=S[:].rearrange("p (t b) -> p t b", b=B),
                                in0=ls_sb[:, sc:sc + tb].unsqueeze(2)
                                    .broadcast_to([128, tb, B]),
                                in1=iota_f[:].unsqueeze(1)
                                    .broadcast_to([128, tb, B]),
                                op=mybir.AluOpType.is_equal,
                            )

                        if l < 2:
                            # feature-major out: [64, bw]
                            hT_ps = psB.tile([H, B], F32, space="PSUM",
                                             tag="hT")
                            for j, slot in enumerate(blkm["tiles"]):
                                nc.tensor.matmul(
                                    out=hT_ps[:, :bw],
                                    lhsT=msg[:, slot * H:(slot + 1) * H],
                                    rhs=S[:, j * B:j * B + bw],
                                    start=(j == 0), stop=False,
                                )
                            nc.tensor.matmul(out=hT_ps[:, :bw], lhsT=wcT[:],
                                             rhs=hTb, start=(tb == 0),
                                             stop=True)
                            hT_sb = bpool.tile([H, B], F32, tag="hT_sb")
                            nc.scalar.activation(hT_sb[:, :bw],
                                                 hT_ps[:, :bw], relu,
                                                 bias=bc[:])
                            nc.sync.dma_start(
                                hT_d[l + 1][:, b * B:b * B + bw],
                                hT_sb[:, :bw])
                            # next-layer premultiplied table slice
                            hw_ps = psD.tile([B, H], F32, space="PSUM",
                                             tag="xw")
                            nc.tensor.matmul(out=hw_ps[:bw, :],
                                             lhsT=hT_sb[:, :bw],
                                             rhs=wlT_nxt[:],
                                             start=True, stop=True)
                            hw_sb = bpool.tile([B, H], F32, tag="hw_sb")
                            nc.scalar.copy(hw_sb[:bw, :], hw_ps[:bw, :])
                            nc.sync.dma_start(
                                slc[l + 1][b * B:b * B + bw, :],
                                hw_sb[:bw, :])
                        else:
                            # node-major final out: [bw, 64]
                            hN_ps = psC.tile([B, H], F32, space="PSUM",
                                             tag="hN")
                            for j, slot in enumerate(blkm["tiles"]):
                                nc.tensor.matmul(
                                    out=hN_ps[:bw, :],
                                    lhsT=S[:, j * B:j * B + bw],
                                    rhs=msg[:, slot * H:(slot + 1) * H],
                                    start=(j == 0), stop=False,
                                )
                            nc.tensor.matmul(out=hN_ps[:bw, :], lhsT=hTb,
                                             rhs=wcT[:], start=(tb == 0),
                                             stop=False)
                            nc.tensor.matmul(out=hN_ps[:bw, :],
                                             lhsT=ones_sb[:, :bw],
                                             rhs=br[:], start=False,
                                             stop=True)
                            hN_sb = bpool.tile([B, H], F32, tag="hN_sb")
                            nc.scalar.activation(hN_sb[:bw, :],
                                                 hN_ps[:bw, :], relu)
                            nc.sync.dma_start(out[b * B:b * B + bw, :],
                                              hN_sb[:bw, :])

                if l < 2:
                    nc.gpsimd.collective_compute(
                        "AllGather", mybir.AluOpType.bypass,
                        ins=[slc[l + 1][:]], outs=[tab[l + 1][:]],
                        replica_groups=rg)

    nc.compile()
    return nc


# --------------------------------------------------------------------------
# public entry point
# --------------------------------------------------------------------------

_CACHE = {}


def _prepare(inputs):
    t0 = time.time()
    meta, per_core = preprocess_edges(np.asarray(inputs["edge_index"]),
                                      np.asarray(inputs["edge_type"]))
    w = prep_weights(inputs)
    x = np.asarray(inputs["x"], np.float32)
    xTf = np.ascontiguousarray(x.T)
    in_maps = []
    for c in range(NC):
        m = {"xT": np.ascontiguousarray(xTf[:, c * NPC:(c + 1) * NPC])}
        m.update(w)
        m.update(per_core[c])
        in_maps.append(m)
    key = tuple(meta["layers"][l]["T"] for l in range(3))
    if key not in _CACHE:
        _CACHE[key] = build_nc(meta)
    return _CACHE[key], in_maps, time.time() - t0


def run(inputs, **kw):
    nc, in_maps, _ = _prepare(inputs)
    res = run_bass_kernel_spmd(nc, in_maps, list(range(NC)), **kw)
    full = np.concatenate([res.results[c]["out"] for c in range(NC)], axis=0)
    return full, res


def kernel(**inputs) -> np.ndarray:
    full, _ = run(inputs)
    return full
